# revision 41
# baseline (speedup 1.0000x reference)
"""Multi-head attention Trainium2 Bass kernel, 8-way SPMD.

Problem: nn_MultiHeadAttention (B=2, S=4096, D=512, H=8, Dk=64), fp32 I/O.

Sharding: the 8192 (B*S) query rows are split into 8 shards of 1024 rows,
one per NeuronCore (core c takes batch c//4, rows (c%4)*1024..). Each core
holds the full key/value sequence of its batch, so there are no
collectives; the per-core output rows concatenate into the full output.

Per-core algorithm (all matmuls contract over the partition axis):
  - PE-transpose raw f32r inputs tile-wise (1.5 cyc/row) to get
    feature-on-partition layouts; the PSUM->SBUF copies downcast to bf16
    so every matmul operand is bf16 (1 cyc/row) and SBUF footprint halves.
  - Q^T = Wq^T.T @ xq^T + bq, K^T = Wk^T.T @ xk^T + bk  (bf16 matmuls,
    fp32 PSUM, bias via DVE tensor_scalar_add, outputs stored bf16).
  - V = xv @ Wv^T + bv stored bf16 with a ones-column appended per head
    (V' = [V_h | 1]), so the AV matmul also produces the softmax
    denominator in PSUM row 64.
  - scores^T[k,q] = lhsT(K^T).T @ Q^T in PSUM; exp via ScalarE with
    scale=1/8 folded in (no max-subtraction: scores ~ N(0,8), exp is safe
    in fp32), output bf16 to SBUF. Head pairs share PE row-groups
    (tile_position auto-derived from base partitions 0/64).
  - attended^T + sums = V'.T @ exp(scores^T)  accumulated over k-chunks.
  - normalize: DVE reciprocal of the PSUM sums row, GpSimd
    partition_broadcast of 1/sum along partitions, one fused DVE multiply
    straight out of PSUM into the bf16 attended^T tile.
  - out = attended^T.T @ Wo^T; bias added by DVE from a GpSimd-broadcast
    bias tile during the PSUM->SBUF copy; one batched DMA per q half.

All DMA loads are batched [128, 4, 512] transfers (the tile framework
multiplexes 8 hardware DMA rings with ~1us completion-semaphore
latency, so DMA instruction count matters more than bytes); x/chunk
loads are issued ahead of the weight stream, and K^T/V' production is
paced chunk-by-chunk into the first attention head-pair loop via three
thunk queues (loads lead transposes lead projections).

repeat>1 builds unroll the whole kernel for delta-timing; timing=True
builds replace the x inputs with device-initialized Internal DRAM so the
per-call staging overhead (~178MB over the axon tunnel) disappears from
measurements.
"""

from contextlib import ExitStack

import numpy as np

B = 2
S = 4096
D = 512
H = 8
DK = 64
P = 128
N_CORES = 8
SQ = (B * S) // N_CORES  # 1024 query rows per core
SKV = S  # 4096 kv rows per core
FC = D // P  # 4 feature chunks
NKT = SKV // P  # 32 key tiles
NSC = SKV // 512  # 8 seq chunks
QH = SQ // 512  # 2 query halves
INV_SCALE = 0.125  # 1/sqrt(DK)

_CACHE = {}


def _build_nc(repeat: int = 1, timing: bool = False, loop: int = 1):
    import concourse.mybir as mybir
    import concourse.tile as tile
    from concourse import bacc
    from concourse.masks import make_identity

    f32 = mybir.dt.float32
    f32r = mybir.dt.float32r
    bf16 = mybir.dt.bfloat16
    EXP = mybir.ActivationFunctionType.Exp

    nc = bacc.Bacc(
        "TRN2",
        target_bir_lowering=False,
        debug=False,
        enable_asserts=False,
        num_devices=N_CORES,
    )

    def din(name, shape):
        kind = "Internal" if timing and name in ("xq", "xk", "xv") else "ExternalInput"
        return nc.dram_tensor(name, shape, f32, kind=kind).ap()

    xq = din("xq", [SQ, D])
    xk = din("xk", [SKV, D])
    xv = din("xv", [SKV, D])
    wq, bq = din("wq", [D, D]), din("bq", [1, D])
    wk, bk = din("wk", [D, D]), din("bk", [1, D])
    wv, bv = din("wv", [D, D]), din("bv", [1, D])
    wo, bo = din("wo", [D, D]), din("bo", [1, D])
    out = nc.dram_tensor("out", [SQ, D], f32, kind="ExternalOutput").ap()

    from contextlib import nullcontext

    with tile.TileContext(nc) as tc:
      if timing:
        # deterministically fill the Internal x tensors (once per call,
        # outside the timing loop)
        with tc.tile_pool(name="init", bufs=1) as initp:
            fill = initp.tile([P, D], f32, name="fill")
            nc.vector.memset(fill, 0.01)
            for t_ap, rows in ((xq, SQ), (xk, SKV), (xv, SKV)):
                for rt in range(rows // P):
                    nc.sync.dma_start(t_ap[rt * P : (rt + 1) * P, :], fill)

      with tc.For_i(0, loop, 1) if loop > 1 else nullcontext():
       for rep in range(repeat):
         sx = f"_r{rep}" if repeat > 1 else ""
         st = ExitStack()
         consts = st.enter_context(tc.tile_pool(name=f"consts{sx}", bufs=1))
         ident = consts.tile([P, P], f32, name=f"ident{sx}")
         make_identity(nc, ident)
         # bias columns [128, 4]: partition = d within tile, col = d-tile
         bcol = {}
         for name, ap in [("bq", bq), ("bk", bk)]:
             t = consts.tile([P, FC], f32, tag=f"bcol_{name}", name=f"bc_{name}{sx}")
             nc.sync.dma_start(t, ap.rearrange("o (t p) -> p (o t)", p=P))
             bcol[name] = t
         # partition-broadcast bias rows [128, 512] for bv / bo (GpSimd)
         bvb = consts.tile([P, D], f32, name=f"bvb{sx}")
         bob = consts.tile([P, D], f32, name=f"bob{sx}")
         with tc.tile_pool(name=f"bstage{sx}", bufs=2) as bstage:
             for name, ap, dst in [("bv", bv, bvb), ("bo", bo, bob)]:
                 stg = bstage.tile([1, D], f32, tag="bstg", name=f"stg_{name}{sx}")
                 nc.sync.dma_start(stg, ap)
                 nc.gpsimd.partition_broadcast(dst, stg)

         # ---- persistent bf16 operand tiles ----
         wT_pool = st.enter_context(tc.tile_pool(name=f"wT{sx}", bufs=1))
         wT = {}
         qT_pool = st.enter_context(tc.tile_pool(name=f"QT{sx}", bufs=1))
         QT = [
             qT_pool.tile([P, SQ], bf16, tag=f"QT{dt}", name=f"QT{dt}{sx}")
             for dt in range(FC)
         ]
         kT_pool = st.enter_context(tc.tile_pool(name=f"KT{sx}", bufs=1))
         KT = [
             kT_pool.tile([P, SKV], bf16, tag=f"KT{dt}", name=f"KT{dt}{sx}")
             for dt in range(FC)
         ]
         vp_pool = st.enter_context(tc.tile_pool(name=f"Vp{sx}", bufs=1))
         Vp = vp_pool.tile([P, NKT, H, DK + 1], bf16, name=f"Vp{sx}")

         # attention pools open before production/head pools so the pool
         # stack stays LIFO (production + head close earlier)
         att_st = ExitStack()
         exp_pool = att_st.enter_context(tc.tile_pool(name=f"exp{sx}", bufs=6))
         spsum = att_st.enter_context(
             tc.tile_pool(name=f"spsum{sx}", bufs=2, space="PSUM")
         )
         attacc = att_st.enter_context(
             tc.tile_pool(name=f"attacc{sx}", bufs=1, space="PSUM")
         )
         norm_pool = att_st.enter_context(tc.tile_pool(name=f"norm{sx}", bufs=2))

         # production pools: staging + transposed chunks + PSUM scratch
         prod_st = ExitStack()
         pxload = prod_st.enter_context(tc.tile_pool(name=f"pxload{sx}", bufs=4))
         xTs_pool = prod_st.enter_context(tc.tile_pool(name=f"xTs{sx}", bufs=4))
         tpsum = prod_st.enter_context(
             tc.tile_pool(name=f"tpsum{sx}", bufs=1, space="PSUM")
         )
         pppsum = prod_st.enter_context(
             tc.tile_pool(name=f"pppsum{sx}", bufs=1, space="PSUM")
         )

         # head pools (weight staging + xq^T; closed once production ends)
         head_st = ExitStack()
         wload = head_st.enter_context(tc.tile_pool(name=f"wload{sx}", bufs=2))
         xqTp = head_st.enter_context(tc.tile_pool(name=f"xqTp{sx}", bufs=1))
         xqT = xqTp.tile([P, FC, SQ], bf16, name=f"xqT{sx}")

         def transpose_tile(dst, src, name):
             """dst[i, fc, 0:128] = src[:, fc*128+i].T via 4 PE transposes
             through one PSUM bank, one DVE copy out (bf16 downcast)."""
             pst = tpsum.tile([P, FC, P], f32, tag="pst", name=name)
             for fc in range(FC):
                 nc.tensor.transpose(
                     pst[:, fc, :], src[:, fc * P : (fc + 1) * P], ident
                 )
             nc.vector.tensor_copy(dst, pst)

         def emit_wT(wname, w_ap):
             wt = wT_pool.tile(
                 [P, FC, D], bf16, tag=f"{wname}T", name=f"{wname}T{sx}"
             )
             wT[wname] = wt
             w_tile = wload.tile(
                 [P, FC, D], f32, tag="wld", name=f"wld_{wname}{sx}"
             )
             nc.sync.dma_start(w_tile, w_ap.rearrange("(rt p) d -> p rt d", p=P))
             for rt in range(FC):
                 transpose_tile(
                     wt[:, :, rt * P : (rt + 1) * P],
                     w_tile[:, rt, :],
                     f"pst_{wname}{rt}{sx}",
                 )

         # ---- production thunk queues: loads / transposes / projections ----
         def chunk_thunks(sc):
             loads, trans_, projs = [], [], []
             for part, x_ap in enumerate([xk, xv]):
                 xTs = xTs_pool.tile(
                     [P, FC, 512], bf16, tag="xTs", name=f"xTs{part}_{sc}{sx}"
                 )
                 x_big = pxload.tile(
                     [P, 4, D], f32, tag="xbig", name=f"xb{part}_{sc}{sx}"
                 )

                 def load(x_ap=x_ap, x_big=x_big):
                     nc.sync.dma_start(
                         x_big,
                         x_ap[sc * 512 : (sc + 1) * 512, :].rearrange(
                             "(rt p) d -> p rt d", p=P
                         ),
                     )

                 loads.append(load)

                 def trans(rt, part=part, x_big=x_big, xTs=xTs):
                     transpose_tile(
                         xTs[:, :, rt * P : (rt + 1) * P],
                         x_big[:, rt, :],
                         f"pstx{part}_{sc}_{rt}{sx}",
                     )

                 for rt in range(4):
                     trans_.append(lambda rt=rt, f=trans: f(rt))
                 if part == 0:

                     def kproj(dt, xTs=xTs):
                         ps = pppsum.tile(
                             [P, 512], f32, tag="pps", name=f"kps{sc}_{dt}{sx}"
                         )
                         for fc in range(FC):
                             nc.tensor.matmul(
                                 ps,
                                 lhsT=wT["wk"][:, fc, dt * P : (dt + 1) * P],
                                 rhs=xTs[:, fc, :],
                                 start=(fc == 0),
                                 stop=(fc == FC - 1),
                             )
                         nc.vector.tensor_scalar_add(
                             KT[dt][:, sc * 512 : (sc + 1) * 512],
                             in0=ps,
                             scalar1=bcol["bk"][:, dt : dt + 1],
                         )

                     for dt in range(FC):
                         projs.append(lambda dt=dt, f=kproj: f(dt))
                 else:

                     def vproj(vt, xTs=xTs):
                         kt = sc * 4 + vt
                         ps = pppsum.tile(
                             [P, 512], f32, tag="pps", name=f"vps{sc}_{vt}{sx}"
                         )
                         for fc in range(FC):
                             nc.tensor.matmul(
                                 ps,
                                 lhsT=xTs[:, fc, vt * P : (vt + 1) * P],
                                 rhs=wT["wv"][:, fc, :],
                                 start=(fc == 0),
                                 stop=(fc == FC - 1),
                             )
                         nc.vector.tensor_add(
                             Vp[:, kt, :, 0:DK],
                             ps.rearrange("p (h d) -> p h d", h=H),
                             bvb.rearrange("p (h d) -> p h d", h=H),
                         )
                         if vt == 3:
                             nc.vector.memset(
                                 Vp[:, sc * 4 : (sc + 1) * 4, :, DK : DK + 1], 1.0
                             )

                     for vt in range(4):
                         projs.append(lambda vt=vt, f=vproj: f(vt))
             return loads, trans_, projs

         def xq_trans_thunks():
             loads, trans_ = [], []
             for ch in range(SQ // 512):
                 x_big = pxload.tile(
                     [P, 4, D], f32, tag="xbig", name=f"xql{ch}{sx}"
                 )

                 def load(ch=ch, x_big=x_big):
                     nc.sync.dma_start(
                         x_big,
                         xq[ch * 512 : (ch + 1) * 512, :].rearrange(
                             "(rt p) d -> p rt d", p=P
                         ),
                     )

                 loads.append(load)

                 def trans(rt, ch=ch, x_big=x_big):
                     st_ = ch * 4 + rt
                     transpose_tile(
                         xqT[:, :, st_ * P : (st_ + 1) * P],
                         x_big[:, rt, :],
                         f"pstq{ch}_{rt}{sx}",
                     )

                 for rt in range(4):
                     trans_.append(lambda rt=rt, f=trans: f(rt))
             return loads, trans_

         LQ, TQ, PQ = [], [], []  # per kv chunk: 2 loads / 8 trans / 8 projs
         lpos = tpos = ppos = 0

         def advance(lt, tt, pt):
             nonlocal lpos, tpos, ppos
             for _ in range(lpos, min(len(LQ), lt)):
                 LQ[lpos]()
                 lpos += 1
             for _ in range(tpos, min(len(TQ), tt)):
                 TQ[tpos]()
                 tpos += 1
             for _ in range(ppos, min(len(PQ), pt)):
                 PQ[ppos]()
                 ppos += 1

         xl_, xt_ = xq_trans_thunks()  # pseudo-chunk: xq (2 loads, 8 trans)
         LQ.extend(xl_)
         TQ.extend(xt_)
         for sc in range(NSC):
             l_, t_, p_ = chunk_thunks(sc)
             LQ.extend(l_)
             TQ.extend(t_)
             PQ.extend(p_)

         # ---- head: queue loads early, transpose, Q-project ----
         advance(4, 0, 0)  # xq + chunk-0 loads queued first
         advance(6, 8, 0)  # chunk-1 loads; xq transposes (first PE work)
         advance(8, 16, 0)  # chunk-2 loads; chunk-0 transposes
         emit_wT("wq", wq)
         # Q projection, with chunk-1 transposes interleaved between groups
         # (separate PSUM banks) so the 1-bank projection serialization
         # doesn't idle the PE
         for qh in range(QH):
             for dt in range(FC):
                 ps = pppsum.tile([P, 512], f32, tag="pps", name=f"qps{dt}{qh}{sx}")
                 for fc in range(FC):
                     nc.tensor.matmul(
                         ps,
                         lhsT=wT["wq"][:, fc, dt * P : (dt + 1) * P],
                         rhs=xqT[:, fc, qh * 512 : (qh + 1) * 512],
                         start=(fc == 0),
                         stop=(fc == FC - 1),
                     )
                 nc.vector.tensor_scalar_add(
                     QT[dt][:, qh * 512 : (qh + 1) * 512],
                     in0=ps,
                     scalar1=bcol["bq"][:, dt : dt + 1],
                 )

         emit_wT("wk", wk)
         emit_wT("wv", wv)
         head_closed = False
         advance(10, 24, 8)  # chunk-3 loads; chunk-0 projections

         # ---- attention + output projection ----
         first_loop = True
         opsum = attT_pool = outbuf = None
         op_st = ExitStack()
         for qh in range(QH):
             qs = slice(qh * 512, (qh + 1) * 512)
             attT_t = None
             for p in range(H // 2):  # head pair (2p, 2p+1)
                 acc = [
                     attacc.tile(
                         [DK + 1, 512], f32, tag=f"acc{i}", name=f"acc{qh}_{p}_{i}{sx}"
                     )
                     for i in range(2)
                 ]
                 for kt in range(NKT):
                     if first_loop:
                         # pace production: loads lead 2 chunks, transposes
                         # 1, projections complete chunk kt//4+1 by the end
                         # of the current 4-kt window
                         c = kt // 4
                         frac = (kt % 4 + 1) / 4
                         advance(
                             2 * (c + 4),
                             8 + int(8 * (c + 2 + frac)),
                             int(8 * (c + 1 + frac)),
                         )
                         if kt == 8 and not head_closed:
                             emit_wT("wo", wo)
                             head_closed = True
                     ks = slice(kt * P, (kt + 1) * P)
                     sc_ps = spsum.tile(
                         [P, 2, 512], f32, tag="sc", name=f"sc{qh}_{p}_{kt}{sx}"
                     )
                     nc.tensor.matmul(
                         sc_ps[:, 0, :],
                         lhsT=KT[p][0:DK, ks],
                         rhs=QT[p][0:DK, qs],
                         start=True,
                         stop=True,
                     )
                     nc.tensor.matmul(
                         sc_ps[:, 1, :],
                         lhsT=KT[p][DK:P, ks],
                         rhs=QT[p][DK:P, qs],
                         start=True,
                         stop=True,
                     )
                     ex = exp_pool.tile(
                         [P, 2, 512], bf16, tag="ex", name=f"ex{qh}_{p}_{kt}{sx}"
                     )
                     nc.scalar.activation(ex, sc_ps, func=EXP, scale=INV_SCALE)
                     for i in range(2):
                         nc.tensor.matmul(
                             acc[i],
                             lhsT=Vp[:, kt, 2 * p + i, :],
                             rhs=ex[:, i, :],
                             start=(kt == 0),
                             stop=(kt == NKT - 1),
                         )
                 if first_loop:
                     # production done; swap production pools for out-proj pools
                     advance(len(LQ), len(TQ), len(PQ))
                     head_st.close()
                     prod_st.close()
                     opsum = op_st.enter_context(
                         tc.tile_pool(name=f"opsum{sx}", bufs=2, space="PSUM")
                     )
                     attT_pool = op_st.enter_context(
                         tc.tile_pool(name=f"attT{sx}", bufs=2)
                     )
                     outbuf = op_st.enter_context(
                         tc.tile_pool(name=f"outbuf{sx}", bufs=2)
                     )
                     first_loop = False
                 if attT_t is None:
                     attT_t = attT_pool.tile(
                         [P, FC, 512], bf16, tag="attT", name=f"attT{qh}{sx}"
                     )
                 for i in range(2):
                     h = 2 * p + i
                     rc = norm_pool.tile(
                         [1, 512], f32, tag="rc", name=f"rc{qh}_{h}{sx}"
                     )
                     nc.vector.reciprocal(rc, acc[i][DK : DK + 1, :])
                     rb = norm_pool.tile(
                         [DK, 512], f32, tag="rb", name=f"rb{qh}_{h}{sx}"
                     )
                     nc.gpsimd.partition_broadcast(rb, rc)
                     nc.vector.tensor_mul(
                         attT_t[(h % 2) * DK : (h % 2 + 1) * DK, h // 2, :],
                         acc[i][0:DK, :],
                         rb,
                     )
             # output projection for this q half
             ot_big = outbuf.tile([P, 4, D], f32, tag="ot", name=f"ot{qh}{sx}")
             for qt in range(4):
                 po = opsum.tile([P, D], f32, tag="po", name=f"po{qh}_{qt}{sx}")
                 for dt in range(FC):
                     nc.tensor.matmul(
                         po,
                         lhsT=attT_t[:, dt, qt * P : (qt + 1) * P],
                         rhs=wT["wo"][:, dt, :],
                         start=(dt == 0),
                         stop=(dt == FC - 1),
                     )
                 nc.vector.tensor_add(ot_big[:, qt, :], po, bob)
             nc.sync.dma_start(
                 out[qh * 512 : (qh + 1) * 512, :].rearrange(
                     "(qt p) d -> p qt d", p=P
                 ),
                 ot_big,
             )
         op_st.close()
         att_st.close()
         st.close()

    nc.compile()
    return nc


def get_nc(repeat: int = 1, timing: bool = False, loop: int = 1):
    key = f"nc{repeat}{'t' if timing else ''}l{loop}"
    if key not in _CACHE:
        _CACHE[key] = _build_nc(repeat, timing, loop)
    return _CACHE[key]


def make_in_maps(query, key, value, w_q, b_q, w_k, b_k, w_v, b_v, w_o, b_o):
    query = np.ascontiguousarray(np.asarray(query, dtype=np.float32)).reshape(
        B * S, D
    )
    key = np.asarray(key, dtype=np.float32)
    value = np.asarray(value, dtype=np.float32)
    shared = {
        "wq": np.ascontiguousarray(w_q, dtype=np.float32),
        "bq": np.ascontiguousarray(b_q, dtype=np.float32).reshape(1, D),
        "wk": np.ascontiguousarray(w_k, dtype=np.float32),
        "bk": np.ascontiguousarray(b_k, dtype=np.float32).reshape(1, D),
        "wv": np.ascontiguousarray(w_v, dtype=np.float32),
        "bv": np.ascontiguousarray(b_v, dtype=np.float32).reshape(1, D),
        "wo": np.ascontiguousarray(w_o, dtype=np.float32),
        "bo": np.ascontiguousarray(b_o, dtype=np.float32).reshape(1, D),
    }
    in_maps = []
    for c in range(N_CORES):
        b = c // (N_CORES // B)
        r0 = (c % (N_CORES // B)) * SQ
        in_maps.append(
            {
                "xq": query[b * S + r0 : b * S + r0 + SQ, :],
                "xk": np.ascontiguousarray(key[b]),
                "xv": np.ascontiguousarray(value[b]),
                **shared,
            }
        )
    return in_maps


def kernel(query, key, value, w_q, b_q, w_k, b_k, w_v, b_v, w_o, b_o):
    from concourse import bass_utils

    in_maps = make_in_maps(
        query, key, value, w_q, b_q, w_k, b_k, w_v, b_v, w_o, b_o
    )
    nc = get_nc()
    res = bass_utils.run_bass_kernel_spmd(nc, in_maps, core_ids=list(range(N_CORES)))
    out = np.concatenate([res.results[c]["out"] for c in range(N_CORES)], axis=0)
    return out.reshape(B, S, D)


if __name__ == "__main__":
    nc = get_nc()
    print("built ok")


# revision 49
# speedup vs baseline: 33.0954x; 33.0954x over previous
"""Multi-head attention Trainium2 Bass kernel, 8-way SPMD.

Problem: nn_MultiHeadAttention (B=2, S=4096, D=512, H=8, Dk=64), fp32 I/O.

Sharding: the 8192 (B*S) query rows are split into 8 shards of 1024 rows,
one per NeuronCore (core c takes batch c//4, rows (c%4)*1024..). Each core
holds the full key/value sequence of its batch, so there are no
collectives; the per-core output rows concatenate into the full output.

Per-core algorithm (all matmuls contract over the partition axis):
  - PE-transpose raw f32r inputs tile-wise (1.5 cyc/row) to get
    feature-on-partition layouts; the PSUM->SBUF copies downcast to bf16
    so every matmul operand is bf16 (1 cyc/row) and SBUF footprint halves.
  - Q^T = Wq^T.T @ xq^T + bq, K^T = Wk^T.T @ xk^T + bk  (bf16 matmuls,
    fp32 PSUM, bias via DVE tensor_scalar_add, outputs stored bf16).
  - V = xv @ Wv^T + bv stored bf16 with a ones-column appended per head
    (V' = [V_h | 1]), so the AV matmul also produces the softmax
    denominator in PSUM row 64.
  - scores^T[k,q] = lhsT(K^T).T @ Q^T in PSUM; exp via ScalarE with
    scale=1/8 folded in (no max-subtraction: scores ~ N(0,8), exp is safe
    in fp32), output bf16 to SBUF. Head pairs share PE row-groups
    (tile_position auto-derived from base partitions 0/64).
  - attended^T + sums = V'.T @ exp(scores^T)  accumulated over k-chunks.
  - normalize: DVE reciprocal of the PSUM sums row, GpSimd
    partition_broadcast of 1/sum along partitions, one fused DVE multiply
    straight out of PSUM into the bf16 attended^T tile.
  - out = attended^T.T @ Wo^T; bias added by DVE from a GpSimd-broadcast
    bias tile during the PSUM->SBUF copy; one batched DMA per q half.

All DMA loads are batched [128, 4, 512] transfers (the tile framework
multiplexes 8 hardware DMA rings with ~1us completion-semaphore
latency, so DMA instruction count matters more than bytes); x/chunk
loads are issued ahead of the weight stream, and K^T/V' production is
paced chunk-by-chunk into the first attention head-pair loop via three
thunk queues (loads lead transposes lead projections).

repeat>1 builds unroll the whole kernel for delta-timing; timing=True
builds replace the x inputs with device-initialized Internal DRAM so the
per-call staging overhead (~178MB over the axon tunnel) disappears from
measurements.
"""

from contextlib import ExitStack

import numpy as np

B = 2
S = 4096
D = 512
H = 8
DK = 64
P = 128
N_CORES = 8
SQ = (B * S) // N_CORES  # 1024 query rows per core
SKV = S  # 4096 kv rows per core
FC = D // P  # 4 feature chunks
NKT = SKV // P  # 32 key tiles
NSC = SKV // 512  # 8 seq chunks
QH = SQ // 512  # 2 query halves
INV_SCALE = 0.125  # 1/sqrt(DK)

_CACHE = {}


def _build_nc(repeat: int = 1, timing: bool = False, loop: int = 1):
    import concourse.mybir as mybir
    import concourse.tile as tile
    from concourse import bacc
    from concourse.masks import make_identity

    f32 = mybir.dt.float32
    f32r = mybir.dt.float32r
    bf16 = mybir.dt.bfloat16
    EXP = mybir.ActivationFunctionType.Exp

    nc = bacc.Bacc(
        "TRN2",
        target_bir_lowering=False,
        debug=False,
        enable_asserts=False,
        num_devices=N_CORES,
    )

    def din(name, shape):
        kind = "Internal" if timing and name in ("xq", "xk", "xv") else "ExternalInput"
        return nc.dram_tensor(name, shape, f32, kind=kind).ap()

    xq = din("xq", [SQ, D])
    xk = din("xk", [SKV, D])
    xv = din("xv", [SKV, D])
    wq, bq = din("wq", [D, D]), din("bq", [1, D])
    wk, bk = din("wk", [D, D]), din("bk", [1, D])
    wv, bv = din("wv", [D, D]), din("bv", [1, D])
    wo, bo = din("wo", [D, D]), din("bo", [1, D])
    out = nc.dram_tensor("out", [SQ, D], f32, kind="ExternalOutput").ap()

    from contextlib import nullcontext

    with tile.TileContext(nc) as tc:
      if timing:
        # deterministically fill the Internal x tensors (once per call,
        # outside the timing loop)
        with tc.tile_pool(name="init", bufs=1) as initp:
            fill = initp.tile([P, D], f32, name="fill")
            nc.vector.memset(fill, 0.01)
            for t_ap, rows in ((xq, SQ), (xk, SKV), (xv, SKV)):
                for rt in range(rows // P):
                    nc.sync.dma_start(t_ap[rt * P : (rt + 1) * P, :], fill)

      with tc.For_i(0, loop, 1) if loop > 1 else nullcontext():
       for rep in range(repeat):
         sx = f"_r{rep}" if repeat > 1 else ""
         st = ExitStack()
         consts = st.enter_context(tc.tile_pool(name=f"consts{sx}", bufs=1))
         ident = consts.tile([P, P], f32, name=f"ident{sx}")
         make_identity(nc, ident)
         # bias columns [128, 4]: partition = d within tile, col = d-tile
         bcol = {}
         for name, ap in [("bq", bq), ("bk", bk)]:
             t = consts.tile([P, FC], f32, tag=f"bcol_{name}", name=f"bc_{name}{sx}")
             nc.sync.dma_start(t, ap.rearrange("o (t p) -> p (o t)", p=P))
             bcol[name] = t
         # partition-broadcast bias rows [128, 512] for bv / bo (GpSimd)
         bvb = consts.tile([P, D], f32, name=f"bvb{sx}")
         bob = consts.tile([P, D], f32, name=f"bob{sx}")
         with tc.tile_pool(name=f"bstage{sx}", bufs=2) as bstage:
             for name, ap, dst in [("bv", bv, bvb), ("bo", bo, bob)]:
                 stg = bstage.tile([1, D], f32, tag="bstg", name=f"stg_{name}{sx}")
                 nc.sync.dma_start(stg, ap)
                 nc.gpsimd.partition_broadcast(dst, stg)

         # ---- persistent bf16 operand tiles ----
         wT_pool = st.enter_context(tc.tile_pool(name=f"wT{sx}", bufs=1))
         wT = {}
         qT_pool = st.enter_context(tc.tile_pool(name=f"QT{sx}", bufs=1))
         QT = [
             qT_pool.tile([P, SQ], bf16, tag=f"QT{dt}", name=f"QT{dt}{sx}")
             for dt in range(FC)
         ]
         kT_pool = st.enter_context(tc.tile_pool(name=f"KT{sx}", bufs=1))
         KT = [
             kT_pool.tile([P, SKV], bf16, tag=f"KT{dt}", name=f"KT{dt}{sx}")
             for dt in range(FC)
         ]
         vp_pool = st.enter_context(tc.tile_pool(name=f"Vp{sx}", bufs=1))
         Vp = vp_pool.tile([P, NKT, H, DK + 1], bf16, name=f"Vp{sx}")

         # attention pools open before production/head pools so the pool
         # stack stays LIFO (production + head close earlier)
         att_st = ExitStack()
         exp_pool = att_st.enter_context(tc.tile_pool(name=f"exp{sx}", bufs=6))
         attacc = att_st.enter_context(
             tc.tile_pool(name=f"attacc{sx}", bufs=1, space="PSUM")
         )
         norm_pool = att_st.enter_context(tc.tile_pool(name=f"norm{sx}", bufs=2))

         # production pools: staging + transposed chunks + PSUM scratch
         prod_st = ExitStack()
         pxload = prod_st.enter_context(tc.tile_pool(name=f"pxload{sx}", bufs=4))
         xTs_pool = prod_st.enter_context(tc.tile_pool(name=f"xTs{sx}", bufs=4))
         spsum = prod_st.enter_context(
             tc.tile_pool(name=f"spsum{sx}", bufs=2, space="PSUM")
         )
         tpsum = prod_st.enter_context(
             tc.tile_pool(name=f"tpsum{sx}", bufs=1, space="PSUM")
         )
         pppsum = prod_st.enter_context(
             tc.tile_pool(name=f"pppsum{sx}", bufs=1, space="PSUM")
         )

         # head pools (weight staging + xq^T; closed once production ends)
         head_st = ExitStack()
         wload = head_st.enter_context(tc.tile_pool(name=f"wload{sx}", bufs=2))
         xqTp = head_st.enter_context(tc.tile_pool(name=f"xqTp{sx}", bufs=1))
         xqT = xqTp.tile([P, FC, SQ], bf16, name=f"xqT{sx}")

         def transpose_tile(dst, src, name):
             """dst[i, fc, 0:128] = src[:, fc*128+i].T via 4 PE transposes
             through one PSUM bank, one DVE copy out (bf16 downcast)."""
             pst = tpsum.tile([P, FC, P], f32, tag="pst", name=name)
             for fc in range(FC):
                 nc.tensor.transpose(
                     pst[:, fc, :], src[:, fc * P : (fc + 1) * P], ident
                 )
             nc.vector.tensor_copy(dst, pst)

         def emit_wT(wname, w_ap):
             wt = wT_pool.tile(
                 [P, FC, D], bf16, tag=f"{wname}T", name=f"{wname}T{sx}"
             )
             wT[wname] = wt
             w_tile = wload.tile(
                 [P, FC, D], f32, tag="wld", name=f"wld_{wname}{sx}"
             )
             nc.sync.dma_start(w_tile, w_ap.rearrange("(rt p) d -> p rt d", p=P))
             for rt in range(FC):
                 transpose_tile(
                     wt[:, :, rt * P : (rt + 1) * P],
                     w_tile[:, rt, :],
                     f"pst_{wname}{rt}{sx}",
                 )

         # ---- production thunk queues: loads / transposes / projections ----
         def chunk_thunks(sc):
             loads, trans_, projs = [], [], []
             for part, x_ap in enumerate([xk, xv]):
                 xTs = xTs_pool.tile(
                     [P, FC, 512], bf16, tag="xTs", name=f"xTs{part}_{sc}{sx}"
                 )
                 x_big = pxload.tile(
                     [P, 4, D], f32, tag="xbig", name=f"xb{part}_{sc}{sx}"
                 )

                 def load(x_ap=x_ap, x_big=x_big):
                     nc.sync.dma_start(
                         x_big,
                         x_ap[sc * 512 : (sc + 1) * 512, :].rearrange(
                             "(rt p) d -> p rt d", p=P
                         ),
                     )

                 loads.append(load)

                 def trans(rt, part=part, x_big=x_big, xTs=xTs):
                     transpose_tile(
                         xTs[:, :, rt * P : (rt + 1) * P],
                         x_big[:, rt, :],
                         f"pstx{part}_{sc}_{rt}{sx}",
                     )

                 for rt in range(4):
                     trans_.append(lambda rt=rt, f=trans: f(rt))
                 if part == 0:

                     def kproj(dt, xTs=xTs):
                         ps = pppsum.tile(
                             [P, 512], f32, tag="pps", name=f"kps{sc}_{dt}{sx}"
                         )
                         for fc in range(FC):
                             nc.tensor.matmul(
                                 ps,
                                 lhsT=wT["wk"][:, fc, dt * P : (dt + 1) * P],
                                 rhs=xTs[:, fc, :],
                                 start=(fc == 0),
                                 stop=(fc == FC - 1),
                             )
                         nc.vector.tensor_scalar_add(
                             KT[dt][:, sc * 512 : (sc + 1) * 512],
                             in0=ps,
                             scalar1=bcol["bk"][:, dt : dt + 1],
                         )

                     for dt in range(FC):
                         projs.append(lambda dt=dt, f=kproj: f(dt))
                 else:

                     def vproj(vt, xTs=xTs):
                         kt = sc * 4 + vt
                         ps = pppsum.tile(
                             [P, 512], f32, tag="pps", name=f"vps{sc}_{vt}{sx}"
                         )
                         for fc in range(FC):
                             nc.tensor.matmul(
                                 ps,
                                 lhsT=xTs[:, fc, vt * P : (vt + 1) * P],
                                 rhs=wT["wv"][:, fc, :],
                                 start=(fc == 0),
                                 stop=(fc == FC - 1),
                             )
                         nc.vector.tensor_add(
                             Vp[:, kt, :, 0:DK],
                             ps.rearrange("p (h d) -> p h d", h=H),
                             bvb.rearrange("p (h d) -> p h d", h=H),
                         )
                         if vt == 3:
                             nc.vector.memset(
                                 Vp[:, sc * 4 : (sc + 1) * 4, :, DK : DK + 1], 1.0
                             )

                     for vt in range(4):
                         projs.append(lambda vt=vt, f=vproj: f(vt))
             return loads, trans_, projs

         def xq_trans_thunks():
             loads, trans_ = [], []
             for ch in range(SQ // 512):
                 x_big = pxload.tile(
                     [P, 4, D], f32, tag="xbig", name=f"xql{ch}{sx}"
                 )

                 def load(ch=ch, x_big=x_big):
                     nc.sync.dma_start(
                         x_big,
                         xq[ch * 512 : (ch + 1) * 512, :].rearrange(
                             "(rt p) d -> p rt d", p=P
                         ),
                     )

                 loads.append(load)

                 def trans(rt, ch=ch, x_big=x_big):
                     st_ = ch * 4 + rt
                     transpose_tile(
                         xqT[:, :, st_ * P : (st_ + 1) * P],
                         x_big[:, rt, :],
                         f"pstq{ch}_{rt}{sx}",
                     )

                 for rt in range(4):
                     trans_.append(lambda rt=rt, f=trans: f(rt))
             return loads, trans_

         LQ, TQ, PQ = [], [], []  # per kv chunk: 2 loads / 8 trans / 8 projs
         lpos = tpos = ppos = 0

         def advance(lt, tt, pt):
             nonlocal lpos, tpos, ppos
             for _ in range(lpos, min(len(LQ), lt)):
                 LQ[lpos]()
                 lpos += 1
             for _ in range(tpos, min(len(TQ), tt)):
                 TQ[tpos]()
                 tpos += 1
             for _ in range(ppos, min(len(PQ), pt)):
                 PQ[ppos]()
                 ppos += 1

         xl_, xt_ = xq_trans_thunks()  # pseudo-chunk: xq (2 loads, 8 trans)
         LQ.extend(xl_)
         TQ.extend(xt_)
         for sc in range(NSC):
             l_, t_, p_ = chunk_thunks(sc)
             LQ.extend(l_)
             TQ.extend(t_)
             PQ.extend(p_)

         # ---- head: queue loads early, transpose, Q-project ----
         advance(4, 0, 0)  # xq + chunk-0 loads queued first
         advance(6, 8, 0)  # chunk-1 loads; xq transposes (first PE work)
         advance(8, 16, 0)  # chunk-2 loads; chunk-0 transposes
         emit_wT("wq", wq)
         # Q projection, with chunk-1 transposes interleaved between groups
         # (separate PSUM banks) so the 1-bank projection serialization
         # doesn't idle the PE
         for qh in range(QH):
             for dt in range(FC):
                 ps = pppsum.tile([P, 512], f32, tag="pps", name=f"qps{dt}{qh}{sx}")
                 for fc in range(FC):
                     nc.tensor.matmul(
                         ps,
                         lhsT=wT["wq"][:, fc, dt * P : (dt + 1) * P],
                         rhs=xqT[:, fc, qh * 512 : (qh + 1) * 512],
                         start=(fc == 0),
                         stop=(fc == FC - 1),
                     )
                 nc.vector.tensor_scalar_add(
                     QT[dt][:, qh * 512 : (qh + 1) * 512],
                     in0=ps,
                     scalar1=bcol["bq"][:, dt : dt + 1],
                 )

         emit_wT("wk", wk)
         emit_wT("wv", wv)
         head_closed = False
         advance(10, 24, 8)  # chunk-3 loads; chunk-0 projections

         # ---- attention + output projection ----
         first_loop = True
         opsum = attT_pool = outbuf = None
         cur_sp = spsum
         op_st = ExitStack()
         for qh in range(QH):
             qs = slice(qh * 512, (qh + 1) * 512)
             attT_t = None
             for p in range(H // 2):  # head pair (2p, 2p+1)
                 acc = [
                     attacc.tile(
                         [DK + 1, 512], f32, tag=f"acc{i}", name=f"acc{qh}_{p}_{i}{sx}"
                     )
                     for i in range(2)
                 ]
                 for kt in range(NKT):
                     if first_loop:
                         # pace production: loads lead 2 chunks, transposes
                         # 1, projections complete chunk kt//4+1 by the end
                         # of the current 4-kt window
                         c = kt // 4
                         frac = (kt % 4 + 1) / 4
                         advance(
                             2 * (c + 4),
                             8 + int(8 * (c + 2 + frac)),
                             int(8 * (c + 1 + frac)),
                         )
                         if kt == 8 and not head_closed:
                             emit_wT("wo", wo)
                             head_closed = True
                     ks = slice(kt * P, (kt + 1) * P)
                     sc_ps = cur_sp.tile(
                         [P, 2, 512], f32, tag="sc", name=f"sc{qh}_{p}_{kt}{sx}"
                     )
                     nc.tensor.matmul(
                         sc_ps[:, 0, :],
                         lhsT=KT[p][0:DK, ks],
                         rhs=QT[p][0:DK, qs],
                         start=True,
                         stop=True,
                     )
                     nc.tensor.matmul(
                         sc_ps[:, 1, :],
                         lhsT=KT[p][DK:P, ks],
                         rhs=QT[p][DK:P, qs],
                         start=True,
                         stop=True,
                     )
                     ex = exp_pool.tile(
                         [P, 2, 512], bf16, tag="ex", name=f"ex{qh}_{p}_{kt}{sx}"
                     )
                     nc.scalar.activation(ex, sc_ps, func=EXP, scale=INV_SCALE)
                     for i in range(2):
                         nc.tensor.matmul(
                             acc[i],
                             lhsT=Vp[:, kt, 2 * p + i, :],
                             rhs=ex[:, i, :],
                             start=(kt == 0),
                             stop=(kt == NKT - 1),
                         )
                 if first_loop:
                     # production done; swap production pools for out-proj pools
                     advance(len(LQ), len(TQ), len(PQ))
                     head_st.close()
                     prod_st.close()
                     # production's PSUM scratch (4 banks incl. the sweep-1
                     # score ring) reopens as a 3-deep score ring: one extra
                     # kt of PE lookahead for the ACT-bound sweeps
                     cur_sp = op_st.enter_context(
                         tc.tile_pool(name=f"spsum2{sx}", bufs=3, space="PSUM")
                     )
                     attT_pool = op_st.enter_context(
                         tc.tile_pool(name=f"attT{sx}", bufs=2)
                     )
                     outbuf = op_st.enter_context(
                         tc.tile_pool(name=f"outbuf{sx}", bufs=2)
                     )
                     first_loop = False
                 if attT_t is None:
                     attT_t = attT_pool.tile(
                         [P, FC, 512], bf16, tag="attT", name=f"attT{qh}{sx}"
                     )
                 for i in range(2):
                     h = 2 * p + i
                     rc = norm_pool.tile(
                         [1, 512], f32, tag="rc", name=f"rc{qh}_{h}{sx}"
                     )
                     nc.vector.reciprocal(rc, acc[i][DK : DK + 1, :])
                     rb = norm_pool.tile(
                         [DK, 512], f32, tag="rb", name=f"rb{qh}_{h}{sx}"
                     )
                     nc.gpsimd.partition_broadcast(rb, rc)
                     nc.vector.tensor_mul(
                         attT_t[(h % 2) * DK : (h % 2 + 1) * DK, h // 2, :],
                         acc[i][0:DK, :],
                         rb,
                     )
             # output projection for this q half
             ot_big = outbuf.tile([P, 4, D], f32, tag="ot", name=f"ot{qh}{sx}")
             for qt in range(4):
                 po = cur_sp.tile(
                     [P, 2, 512], f32, tag="sc", name=f"po{qh}_{qt}{sx}"
                 )
                 for dt in range(FC):
                     nc.tensor.matmul(
                         po[:, 0, :],
                         lhsT=attT_t[:, dt, qt * P : (qt + 1) * P],
                         rhs=wT["wo"][:, dt, :],
                         start=(dt == 0),
                         stop=(dt == FC - 1),
                     )
                 nc.vector.tensor_add(ot_big[:, qt, :], po[:, 0, :], bob)
             nc.sync.dma_start(
                 out[qh * 512 : (qh + 1) * 512, :].rearrange(
                     "(qt p) d -> p qt d", p=P
                 ),
                 ot_big,
             )
         op_st.close()
         att_st.close()
         st.close()

    nc.compile()
    return nc


def get_nc(repeat: int = 1, timing: bool = False, loop: int = 1):
    key = f"nc{repeat}{'t' if timing else ''}l{loop}"
    if key not in _CACHE:
        _CACHE[key] = _build_nc(repeat, timing, loop)
    return _CACHE[key]


def make_in_maps(query, key, value, w_q, b_q, w_k, b_k, w_v, b_v, w_o, b_o):
    query = np.ascontiguousarray(np.asarray(query, dtype=np.float32)).reshape(
        B * S, D
    )
    key = np.asarray(key, dtype=np.float32)
    value = np.asarray(value, dtype=np.float32)
    shared = {
        "wq": np.ascontiguousarray(w_q, dtype=np.float32),
        "bq": np.ascontiguousarray(b_q, dtype=np.float32).reshape(1, D),
        "wk": np.ascontiguousarray(w_k, dtype=np.float32),
        "bk": np.ascontiguousarray(b_k, dtype=np.float32).reshape(1, D),
        "wv": np.ascontiguousarray(w_v, dtype=np.float32),
        "bv": np.ascontiguousarray(b_v, dtype=np.float32).reshape(1, D),
        "wo": np.ascontiguousarray(w_o, dtype=np.float32),
        "bo": np.ascontiguousarray(b_o, dtype=np.float32).reshape(1, D),
    }
    in_maps = []
    for c in range(N_CORES):
        b = c // (N_CORES // B)
        r0 = (c % (N_CORES // B)) * SQ
        in_maps.append(
            {
                "xq": query[b * S + r0 : b * S + r0 + SQ, :],
                "xk": np.ascontiguousarray(key[b]),
                "xv": np.ascontiguousarray(value[b]),
                **shared,
            }
        )
    return in_maps


def kernel(query, key, value, w_q, b_q, w_k, b_k, w_v, b_v, w_o, b_o):
    from concourse import bass_utils

    in_maps = make_in_maps(
        query, key, value, w_q, b_q, w_k, b_k, w_v, b_v, w_o, b_o
    )
    nc = get_nc()
    res = bass_utils.run_bass_kernel_spmd(nc, in_maps, core_ids=list(range(N_CORES)))
    out = np.concatenate([res.results[c]["out"] for c in range(N_CORES)], axis=0)
    return out.reshape(B, S, D)


if __name__ == "__main__":
    nc = get_nc()
    print("built ok")
